# revision 2
# baseline (speedup 1.0000x reference)
"""Pairwise distance screen (CellList) kernel for 8 Trainium2 NeuronCores.

Computes the masked dense [N, N] lower-triangular distance matrix:
  out[i, j] = sqrt(|c_i - c_j|^2)  if  j < i, both species valid, d2 <= cutoff^2
            = 0                    otherwise
with d2 evaluated with exactly the same f32 operation order as the reference
(diff -> square -> sum), so the cutoff mask decisions match bit for bit.

Strategy:
  - Rows are split into 48 blocks of 128. Core c owns blocks
    sorted([c, c+8, c+16, 31-c, 39-c, 47-c]); slot r of every core is padded
    to WMAX[r] = 8*(r+1) col-blocks so all 8 cores share one SPMD program.
    Columns past a core's true diagonal are zeroed by the tril mask; columns
    past the padded width are never written (outputs are donated zero
    buffers).
  - Coordinates are broadcast along partitions bit-exactly by the tensor
    engine: x = xh + xm + xl (exact 3-way bf16 split), K=3 ones-matmul
    accumulated in fp32 PSUM.
  - DVE custom op SQDIFF2 computes (xj-xi)^2 + (yj-yi)^2 in one pass;
    custom op SCREEN_DZ adds dz^2, applies the tril mask (Idx scan vs
    per-partition threshold) and the cutoff compare (t < nextafter(cut2)
    == t <= cut2), and selects t or 0.
  - ACT computes dz = zB - zi (Identity w/ bias) and the final sqrt;
    sqrt(0) = 0 keeps masked entries at zero.
"""

import threading

import numpy as np

N = 6144
P = 128
NCORES = 8
BAND = 512
NBANDS = N // BAND  # 12
WMAX = [8, 16, 24, 32, 40, 48]  # padded slot widths in 128-col blocks

_lock = threading.Lock()
_cache: dict = {}


def _blocks_for_core(c: int) -> list[int]:
    return sorted([c, c + 8, c + 16, 31 - c, 39 - c, 47 - c])


def _chunk_schedule():
    """Emission order of (band, slot) chunks; slot r active iff band < 2*(r+1)."""
    sched = []
    for k in range(NBANDS):
        for r in range(6):
            if k < 2 * (r + 1):
                sched.append((k, r))
    return sched


def _register_ops():
    """Register the two fused DVE ops at runtime (visible to table-gen)."""
    import concourse.dve_ops as dve_ops
    from concourse.dve_spec import (
        C0,
        C1,
        Idx,
        Spec,
        Src0,
        Src1,
        Zero,
        _has_src1,
        lower,
        select,
        sq,
    )
    from concourse.dve_uop import DveOpSpec

    def make(name, body, ref):
        for op in dve_ops.OPS:
            if op.name == name:
                return op
        spec = Spec(body=body, reference=ref)
        row = 1 + len(dve_ops.OPS)
        assert row < 0x20
        shas = {}
        for ver in ("v3", "v4"):
            uops = lower(spec, ver=ver)
            shas[ver] = DveOpSpec(
                name=name, opcode=row, uops=uops, rd1_en=_has_src1(spec)
            ).sha(ver)
        op = dve_ops.DveOp(name, spec, subdim=False, uops_sha=shas)
        dve_ops._SUB_OPCODE_FOR_NAME[name] = row
        dve_ops.OPS.append(op)
        dve_ops.CUSTOM_DVE_SPECS[name] = spec
        return op

    # out = (in0 - s0)^2 + (in1 - s1)^2
    sqdiff2 = make(
        "SQDIFF2_ANT",
        sq(Src0 - C0) + sq(Src1 - C1),
        lambda in0, in1, s0, s1, imm2: (
            (in0.astype(np.float32) - s0) ** 2 + (in1.astype(np.float32) - s1) ** 2
        ).astype(np.float32),
    )

    # t = in0^2 + in1 ; out = (Idx < s0) & (t < s1) ? t : 0
    def screen_ref(in0, in1, s0, s1, imm2):
        t = (in0.astype(np.float32) ** 2 + in1.astype(np.float32)).astype(np.float32)
        idx = np.arange(t.shape[-1], dtype=np.float32)[None, :]
        keep = (idx < s0) & (t < s1)
        return np.where(keep, t, 0.0).astype(np.float32)

    t = sq(Src0) + Src1
    screen = make(
        "SCREEN_DZ_ANT",
        select((Idx < C0) & (t < C1), t, Zero),
        screen_ref,
    )
    return sqdiff2, screen


def _build_program():
    import concourse.bacc as bacc
    import concourse.mybir as mybir
    import concourse.tile as tile

    sqdiff2, screen = _register_ops()

    nc = bacc.Bacc("TRN2", target_bir_lowering=False, debug=False, num_devices=NCORES)
    f32 = mybir.dt.float32
    bf16 = mybir.dt.bfloat16
    Ident = mybir.ActivationFunctionType.Identity

    splits = nc.dram_tensor("splits", [3, 3 * N], bf16, kind="ExternalInput")
    xi6 = nc.dram_tensor("xi6", [P, 6], f32, kind="ExternalInput")
    yi6 = nc.dram_tensor("yi6", [P, 6], f32, kind="ExternalInput")
    nzi6 = nc.dram_tensor("nzi6", [P, 6], f32, kind="ExternalInput")
    cuthi = nc.dram_tensor("cuthi", [P, 1], f32, kind="ExternalInput")
    idxthr = nc.dram_tensor("idxthr", [P, 42], f32, kind="ExternalInput")
    out = nc.dram_tensor("out", [6 * P, N], f32, kind="ExternalOutput")

    sched = _chunk_schedule()

    with tile.TileContext(nc) as tc:
        with (
            tc.tile_pool(name="const", bufs=1) as cpool,
            tc.tile_pool(name="work", bufs=4) as wpool,
            tc.tile_pool(name="psum", bufs=2, space="PSUM") as ppool,
        ):
            splits_t = cpool.tile([3, 3 * N], bf16, tag="splits")
            ones_t = cpool.tile([3, P], bf16, tag="ones")
            xi_t = cpool.tile([P, 6], f32, tag="xi")
            yi_t = cpool.tile([P, 6], f32, tag="yi")
            nzi_t = cpool.tile([P, 6], f32, tag="nzi")
            cut_t = cpool.tile([P, 1], f32, tag="cut")
            ithr_t = cpool.tile([P, 42], f32, tag="ithr")

            nc.sync.dma_start(splits_t[:], splits[:])
            nc.sync.dma_start(xi_t[:], xi6[:])
            nc.sync.dma_start(yi_t[:], yi6[:])
            nc.sync.dma_start(nzi_t[:], nzi6[:])
            nc.sync.dma_start(cut_t[:], cuthi[:])
            nc.sync.dma_start(ithr_t[:], idxthr[:])
            nc.gpsimd.memset(ones_t[:], 1.0)

            cnt = 0
            last_k = -1
            xb = yb = zb = ycp = None
            for k, r in sched:
                if k != last_k:
                    last_k = k
                    c0 = k * BAND
                    xb = ppool.tile([P, BAND], f32, tag="xb")
                    yb = ppool.tile([P, BAND], f32, tag="yb")
                    zb = ppool.tile([P, BAND], f32, tag="zb")
                    for tile_, coord in ((xb, 0), (yb, 1), (zb, 2)):
                        nc.tensor.matmul(
                            tile_[:],
                            ones_t[:],
                            splits_t[:, coord * N + c0 : coord * N + c0 + BAND],
                            start=True,
                            stop=True,
                        )
                    ycp = wpool.tile([P, BAND], f32, tag="ycp")
                    nc.scalar.copy(ycp[:], yb[:])

                c0 = k * BAND
                dz = wpool.tile([P, BAND], f32, tag="dz")
                nc.scalar.activation(
                    dz[:], zb[:], Ident, bias=nzi_t[:, r : r + 1], scale=1.0
                )
                dxy2 = wpool.tile([P, BAND], f32, tag="dxy2")
                nc.vector._custom_dve(
                    sqdiff2,
                    out=dxy2[:],
                    in0=xb[:],
                    in1=ycp[:],
                    s0=xi_t[:, r : r + 1],
                    s1=yi_t[:, r : r + 1],
                )
                v = wpool.tile([P, BAND], f32, tag="v")
                nc.vector._custom_dve(
                    screen,
                    out=v[:],
                    in0=dz[:],
                    in1=dxy2[:],
                    s0=ithr_t[:, cnt : cnt + 1],
                    s1=cut_t[:],
                )
                s = wpool.tile([P, BAND], f32, tag="s")
                nc.scalar.sqrt(s[:], v[:])
                nc.sync.dma_start(out[r * P : (r + 1) * P, c0 : c0 + BAND], s[:])
                cnt += 1

    nc.compile()
    return nc


def _get_program():
    with _lock:
        if "nc" not in _cache:
            _cache["nc"] = _build_program()
    return _cache["nc"]


def _split3_bf16(v32: np.ndarray):
    """Exact 3-way bf16 split: v32 == hi + mid + lo (as f32 sums, any order)."""
    import ml_dtypes

    bf = ml_dtypes.bfloat16
    hi = v32.astype(bf)
    r1 = (v32 - hi.astype(np.float32)).astype(np.float32)
    mid = r1.astype(bf)
    lo = (r1 - mid.astype(np.float32)).astype(np.float32).astype(bf)
    # verify exactness (cheap); required for the bit-exact mask
    recon = (
        hi.astype(np.float32) + mid.astype(np.float32) + lo.astype(np.float32)
    ).astype(np.float32)
    assert np.array_equal(recon, v32), "bf16 3-way split not exact"
    return hi, mid, lo


def _prepare_inputs(species, coordinates, cutoff):
    coords = np.asarray(coordinates, dtype=np.float32).reshape(-1, 3).copy()
    assert coords.shape[0] == N
    valid = np.asarray(species).reshape(-1) >= 0
    if not valid.all():
        bad = np.where(~valid)[0]
        coords[bad] = (1.0e5 + 1.0e4 * np.arange(len(bad), dtype=np.float32))[:, None]

    x, y, z = coords[:, 0].copy(), coords[:, 1].copy(), coords[:, 2].copy()

    splits = np.empty((3, 3 * N), dtype=np.float32)
    for ci, v in enumerate((x, y, z)):
        hi, mid, lo = _split3_bf16(v)
        splits[0, ci * N : (ci + 1) * N] = hi.astype(np.float32)
        splits[1, ci * N : (ci + 1) * N] = mid.astype(np.float32)
        splits[2, ci * N : (ci + 1) * N] = lo.astype(np.float32)
    import ml_dtypes

    splits = splits.astype(ml_dtypes.bfloat16)

    cut2 = np.float32(cutoff) * np.float32(cutoff)
    cut_hi = np.nextafter(cut2, np.float32(np.inf), dtype=np.float32)
    cuthi = np.full((P, 1), cut_hi, np.float32)

    sched = _chunk_schedule()
    in_maps = []
    for c in range(NCORES):
        blocks = _blocks_for_core(c)
        rows = np.concatenate([np.arange(P * b, P * b + P) for b in blocks])
        rmat = rows.reshape(6, P)  # [slot, partition]
        xi6 = np.ascontiguousarray(x[rmat].T)  # [128, 6]
        yi6 = np.ascontiguousarray(y[rmat].T)
        nzi6 = np.ascontiguousarray(-z[rmat].T)
        idxthr = np.empty((P, len(sched)), np.float32)
        for cnt, (k, r) in enumerate(sched):
            idxthr[:, cnt] = rmat[r].astype(np.float32) - np.float32(k * BAND)
        in_maps.append(
            {
                "splits": splits,
                "xi6": xi6,
                "yi6": yi6,
                "nzi6": nzi6,
                "cuthi": cuthi,
                "idxthr": idxthr,
            }
        )
    return in_maps


def _run(in_maps, trace=False):
    from concourse import bass_utils

    nc = _get_program()
    return bass_utils.run_bass_kernel_spmd(
        nc, in_maps, core_ids=list(range(NCORES)), trace=trace
    )


def _assemble(results):
    full = np.zeros((N, N), np.float32)
    for c in range(NCORES):
        o = results[c]["out"]
        for r, b in enumerate(_blocks_for_core(c)):
            full[P * b : P * (b + 1), :] = o[P * r : P * (r + 1), :]
    return full


def kernel(species, coordinates, cutoff):
    in_maps = _prepare_inputs(species, coordinates, cutoff)
    res = _run(in_maps)
    return _assemble(res.results)


# revision 6
# speedup vs baseline: 1.1041x; 1.1041x over previous
"""Pairwise distance screen (CellList) kernel for 8 Trainium2 NeuronCores.

Computes the masked dense [N, N] lower-triangular distance matrix:
  out[i, j] = sqrt(|c_i - c_j|^2)  if  j < i, both species valid, d2 <= cutoff^2
            = 0                    otherwise
with d2 evaluated with exactly the same f32 operation order as the reference
(diff -> square -> sum), so the cutoff mask decisions match bit for bit.

Strategy:
  - Rows are split into 48 blocks of 128. Core c owns blocks
    sorted([c, c+8, c+16, 31-c, 39-c, 47-c]); slot r of every core is padded
    to WMAX[r] = 8*(r+1) col-blocks so all 8 cores share one SPMD program.
    Columns past a core's true diagonal are zeroed by the tril mask; columns
    past the padded width are never written (outputs are donated zero
    buffers).
  - Coordinates are broadcast along partitions bit-exactly by the tensor
    engine: x = xh + xm + xl (exact 3-way bf16 split), K=3 ones-matmul
    accumulated in fp32 PSUM.
  - DVE custom op SQDIFF2 computes (xj-xi)^2 + (yj-yi)^2 in one pass;
    custom op SCREEN_DZ adds dz^2, applies the tril mask (Idx scan vs
    per-partition threshold) and the cutoff compare (t < nextafter(cut2)
    == t <= cut2), and selects t or 0.
  - ACT computes dz = zB - zi (Identity w/ bias) and the final sqrt;
    sqrt(0) = 0 keeps masked entries at zero.
"""

import threading

import numpy as np

N = 6144
P = 128
NCORES = 8
BAND = 1024
NBANDS = N // BAND  # 6
MMW = 512  # matmul free-dim width (one PSUM bank)

_lock = threading.Lock()
_cache: dict = {}


def _blocks_for_core(c: int) -> list[int]:
    return sorted([c, c + 8, c + 16, 31 - c, 39 - c, 47 - c])


def _chunk_schedule():
    """Emission order of (band, slot) chunks; slot r active iff band <= r."""
    sched = []
    for k in range(NBANDS):
        for r in range(6):
            if k <= r:
                sched.append((k, r))
    return sched


def _register_ops():
    """Register the two fused DVE ops at runtime (visible to table-gen)."""
    import concourse.dve_ops as dve_ops
    from concourse.dve_spec import (
        C0,
        C1,
        Idx,
        Spec,
        Src0,
        Src1,
        Zero,
        _has_src1,
        lower,
        select,
        sq,
    )
    from concourse.dve_uop import DveOpSpec

    def make(name, body, ref):
        for op in dve_ops.OPS:
            if op.name == name:
                return op
        spec = Spec(body=body, reference=ref)
        row = 1 + len(dve_ops.OPS)
        assert row < 0x20
        shas = {}
        for ver in ("v3", "v4"):
            uops = lower(spec, ver=ver)
            shas[ver] = DveOpSpec(
                name=name, opcode=row, uops=uops, rd1_en=_has_src1(spec)
            ).sha(ver)
        op = dve_ops.DveOp(name, spec, subdim=False, uops_sha=shas)
        dve_ops._SUB_OPCODE_FOR_NAME[name] = row
        dve_ops.OPS.append(op)
        dve_ops.CUSTOM_DVE_SPECS[name] = spec
        return op

    # out = (in0 - s0)^2 + (in1 - s1)^2
    sqdiff2 = make(
        "SQDIFF2_ANT",
        sq(Src0 - C0) + sq(Src1 - C1),
        lambda in0, in1, s0, s1, imm2: (
            (in0.astype(np.float32) - s0) ** 2 + (in1.astype(np.float32) - s1) ** 2
        ).astype(np.float32),
    )

    # t = in0^2 + in1 ; out = (Idx < s0) & (t < s1) ? t : 0
    def screen_ref(in0, in1, s0, s1, imm2):
        t = (in0.astype(np.float32) ** 2 + in1.astype(np.float32)).astype(np.float32)
        idx = np.arange(t.shape[-1], dtype=np.float32)[None, :]
        keep = (idx < s0) & (t < s1)
        return np.where(keep, t, 0.0).astype(np.float32)

    t = sq(Src0) + Src1
    screen = make(
        "SCREEN_DZ_ANT",
        select((Idx < C0) & (t < C1), t, Zero),
        screen_ref,
    )
    return sqdiff2, screen


def _build_program():
    import concourse.bacc as bacc
    import concourse.mybir as mybir
    import concourse.tile as tile

    sqdiff2, screen = _register_ops()

    nc = bacc.Bacc("TRN2", target_bir_lowering=False, debug=False, num_devices=NCORES)
    f32 = mybir.dt.float32
    bf16 = mybir.dt.bfloat16
    Ident = mybir.ActivationFunctionType.Identity

    splits = nc.dram_tensor("splits", [3, 3 * N], bf16, kind="ExternalInput")
    xi6 = nc.dram_tensor("xi6", [P, 6], f32, kind="ExternalInput")
    yi6 = nc.dram_tensor("yi6", [P, 6], f32, kind="ExternalInput")
    nzi6 = nc.dram_tensor("nzi6", [P, 6], f32, kind="ExternalInput")
    cuthi = nc.dram_tensor("cuthi", [P, 1], f32, kind="ExternalInput")
    idxthr = nc.dram_tensor("idxthr", [P, 21], f32, kind="ExternalInput")
    out = nc.dram_tensor("out", [6 * P, N], f32, kind="ExternalOutput")

    sched = _chunk_schedule()

    with tile.TileContext(nc) as tc:
        with (
            tc.tile_pool(name="const", bufs=1) as cpool,
            tc.tile_pool(name="work", bufs=4) as wpool,
            tc.tile_pool(name="bandw", bufs=2) as bpool,
            tc.tile_pool(name="psx", bufs=2, space="PSUM") as ppx,
            tc.tile_pool(name="psy", bufs=1, space="PSUM") as ppy,
            tc.tile_pool(name="psz", bufs=1, space="PSUM") as ppz,
        ):
            splits_t = cpool.tile([3, 3 * N], bf16, tag="splits")
            ones_t = cpool.tile([3, P], bf16, tag="ones")
            xi_t = cpool.tile([P, 6], f32, tag="xi")
            yi_t = cpool.tile([P, 6], f32, tag="yi")
            nzi_t = cpool.tile([P, 6], f32, tag="nzi")
            cut_t = cpool.tile([P, 1], f32, tag="cut")
            ithr_t = cpool.tile([P, 21], f32, tag="ithr")

            nc.sync.dma_start(splits_t[:], splits[:])
            nc.sync.dma_start(xi_t[:], xi6[:])
            nc.sync.dma_start(yi_t[:], yi6[:])
            nc.sync.dma_start(nzi_t[:], nzi6[:])
            nc.sync.dma_start(cut_t[:], cuthi[:])
            nc.sync.dma_start(ithr_t[:], idxthr[:])
            nc.gpsimd.memset(ones_t[:], 1.0)

            def bcast(tile_, coord, c0):
                for h in range(0, BAND, MMW):
                    nc.tensor.matmul(
                        tile_[:, h : h + MMW],
                        ones_t[:],
                        splits_t[:, coord * N + c0 + h : coord * N + c0 + h + MMW],
                        start=True,
                        stop=True,
                    )

            cnt = 0
            last_k = -1
            xb = yb = zb = ycp = None
            dzs = {}
            for k, r in sched:
                c0 = k * BAND
                if k != last_k:
                    last_k = k
                    yb = ppy.tile([P, BAND], f32, tag="yb")
                    bcast(yb, 1, c0)
                    ycp = bpool.tile([P, BAND], f32, tag="ycp")
                    nc.scalar.copy(ycp[:], yb[:])
                    zb = ppz.tile([P, BAND], f32, tag="zb")
                    bcast(zb, 2, c0)
                    # all dz for this band up front so zb is released early
                    dzs = {}
                    for r2 in range(k, 6):
                        dz = bpool.tile([P, BAND], f32, tag=f"dz{r2}")
                        nc.scalar.activation(
                            dz[:], zb[:], Ident, bias=nzi_t[:, r2 : r2 + 1], scale=1.0
                        )
                        dzs[r2] = dz
                    xb = ppx.tile([P, BAND], f32, tag="xb")
                    bcast(xb, 0, c0)

                dxy2 = wpool.tile([P, BAND], f32, tag="dxy2")
                nc.vector._custom_dve(
                    sqdiff2,
                    out=dxy2[:],
                    in0=xb[:],
                    in1=ycp[:],
                    s0=xi_t[:, r : r + 1],
                    s1=yi_t[:, r : r + 1],
                )
                v = wpool.tile([P, BAND], f32, tag="v")
                nc.vector._custom_dve(
                    screen,
                    out=v[:],
                    in0=dzs[r][:],
                    in1=dxy2[:],
                    s0=ithr_t[:, cnt : cnt + 1],
                    s1=cut_t[:],
                )
                s = wpool.tile([P, BAND], f32, tag="s")
                nc.scalar.sqrt(s[:], v[:])
                nc.sync.dma_start(out[r * P : (r + 1) * P, c0 : c0 + BAND], s[:])
                cnt += 1

    nc.compile()
    return nc


def _get_program():
    with _lock:
        if "nc" not in _cache:
            _cache["nc"] = _build_program()
    return _cache["nc"]


def _split3_bf16(v32: np.ndarray):
    """Exact 3-way bf16 split: v32 == hi + mid + lo (as f32 sums, any order)."""
    import ml_dtypes

    bf = ml_dtypes.bfloat16
    hi = v32.astype(bf)
    r1 = (v32 - hi.astype(np.float32)).astype(np.float32)
    mid = r1.astype(bf)
    lo = (r1 - mid.astype(np.float32)).astype(np.float32).astype(bf)
    # verify exactness (cheap); required for the bit-exact mask
    recon = (
        hi.astype(np.float32) + mid.astype(np.float32) + lo.astype(np.float32)
    ).astype(np.float32)
    assert np.array_equal(recon, v32), "bf16 3-way split not exact"
    return hi, mid, lo


def _prepare_inputs(species, coordinates, cutoff):
    coords = np.asarray(coordinates, dtype=np.float32).reshape(-1, 3).copy()
    assert coords.shape[0] == N
    valid = np.asarray(species).reshape(-1) >= 0
    if not valid.all():
        bad = np.where(~valid)[0]
        coords[bad] = (1.0e5 + 1.0e4 * np.arange(len(bad), dtype=np.float32))[:, None]

    x, y, z = coords[:, 0].copy(), coords[:, 1].copy(), coords[:, 2].copy()

    splits = np.empty((3, 3 * N), dtype=np.float32)
    for ci, v in enumerate((x, y, z)):
        hi, mid, lo = _split3_bf16(v)
        splits[0, ci * N : (ci + 1) * N] = hi.astype(np.float32)
        splits[1, ci * N : (ci + 1) * N] = mid.astype(np.float32)
        splits[2, ci * N : (ci + 1) * N] = lo.astype(np.float32)
    import ml_dtypes

    splits = splits.astype(ml_dtypes.bfloat16)

    cut2 = np.float32(cutoff) * np.float32(cutoff)
    cut_hi = np.nextafter(cut2, np.float32(np.inf), dtype=np.float32)
    cuthi = np.full((P, 1), cut_hi, np.float32)

    sched = _chunk_schedule()
    in_maps = []
    for c in range(NCORES):
        blocks = _blocks_for_core(c)
        rows = np.concatenate([np.arange(P * b, P * b + P) for b in blocks])
        rmat = rows.reshape(6, P)  # [slot, partition]
        xi6 = np.ascontiguousarray(x[rmat].T)  # [128, 6]
        yi6 = np.ascontiguousarray(y[rmat].T)
        nzi6 = np.ascontiguousarray(-z[rmat].T)
        idxthr = np.empty((P, len(sched)), np.float32)
        for cnt, (k, r) in enumerate(sched):
            idxthr[:, cnt] = rmat[r].astype(np.float32) - np.float32(k * BAND)
        in_maps.append(
            {
                "splits": splits,
                "xi6": xi6,
                "yi6": yi6,
                "nzi6": nzi6,
                "cuthi": cuthi,
                "idxthr": idxthr,
            }
        )
    return in_maps


def _run(in_maps, trace=False):
    from concourse import bass_utils

    nc = _get_program()
    return bass_utils.run_bass_kernel_spmd(
        nc, in_maps, core_ids=list(range(NCORES)), trace=trace
    )


def _assemble(results):
    full = np.zeros((N, N), np.float32)
    for c in range(NCORES):
        o = results[c]["out"]
        for r, b in enumerate(_blocks_for_core(c)):
            full[P * b : P * (b + 1), :] = o[P * r : P * (r + 1), :]
    return full


def kernel(species, coordinates, cutoff):
    in_maps = _prepare_inputs(species, coordinates, cutoff)
    res = _run(in_maps)
    return _assemble(res.results)
